# revision 1
# baseline (speedup 1.0000x reference)
"""Trainium2 Bass kernel for nn_DEFNet: 16-branch 1D conv (k=3..33) + bias + ReLU
+ channel-mean over x[32, 1, 262144] -> out[32, 262144].

Strategy (per core, 8 cores, 4 batch rows each):
  - Host builds a transposed sliding-window view xwinT[k, t] = xpad[64t + k]
    (k in [0,96)) so each channel-pair's conv is ONE matmul:
       out[(c,p), t] = sum_k lhsT[k, 64c+p] * xwinT[k, t],  p in [0,64)
    with lhsT[k, 64c+p] = w_masked[2j+c, k-p] / 16 (mean folded into weights).
  - 8 matmuls (channel pairs) -> f32 PSUM; 4 pairs relu'd on ScalarE
    (bias fused) and tree-summed (bf16) on GpSimd/VectorE; 4 pairs run as a
    fused max-add chain on VectorE reading PSUM directly. The [128, T] sum
    tile is DMA'd out position-major; the host folds the two 64-row halves,
    adds the chain-bias constant, and transposes back to natural order.
"""

import os

import numpy as np

import concourse.bass as bass
import concourse.mybir as mybir
import concourse.tile as tile
from concourse import bacc, bass_utils
from concourse.tile import TileContext

B, L = 32, 262144
NCONV, MAXK = 16, 33
NCORES = 8
ROWS = B // NCORES          # batch rows per core
P = 64                      # output positions per segment
W = 96                      # window rows (matmul contraction dim)
HALO = 16
T = L // P                  # segments per row (4096)

# --- tunables -------------------------------------------------------------
CONV_DT = os.environ.get("KERNEL_CONV_DT", "fp16")   # "fp16" | "fp32r"
BLK = 1024                  # segments per block (psum tile free dim)
SB_SEG = 2048               # segments per super-block
# pairs relu'd on ScalarE (separate tiles, tree-added on gpsimd);
# remaining pairs run as a fused max-add chain on VectorE (reads PSUM).
RELU_ACT_PAIRS = (0, 1, 2, 3)

# matmul output is fp32 in PSUM; one matmul <= one 2KB bank -> MMN <= 512
_DT = {
    "fp16": (mybir.dt.float16, mybir.dt.float16, 512),
    "fp32r": (mybir.dt.float32, mybir.dt.float32, 512),
}
DT_X, DT_W, MMN = _DT[CONV_DT]
DT_E = mybir.dt.bfloat16    # relu/accumulate dtype (measured TT: bf16 1.43us < fp16 2.5 ~ fp32 2.8)
F32 = mybir.dt.float32


def _support_mask():
    m = np.zeros((NCONV, MAXK), dtype=np.float32)
    c = MAXK // 2
    for i in range(1, NCONV + 1):
        m[i - 1, c - i:c + i + 1] = 1.0
    return m


def _build_lhsT(w):
    """[96, 8*128] f32; pair j cols j*128..(j+1)*128, lhsT[k, 64c+p] = wm[2j+c, k-p]/16."""
    wm = (np.asarray(w, np.float32) * _support_mask()) / 16.0
    lhsT = np.zeros((W, 8 * 128), dtype=np.float32)
    # lhsT[p+d, 64c+p] = wm[ch, d]
    for j in range(8):
        for c in range(2):
            ch = 2 * j + c
            for p in range(P):
                lhsT[p:p + MAXK, j * 128 + c * 64 + p] = wm[ch]
    return lhsT


def _build_nc():
    nc = bacc.Bacc(
        "TRN2",
        target_bir_lowering=False,
        debug=False,
        enable_asserts=False,
        num_devices=NCORES,
    )
    xwin = nc.dram_tensor("xwin", [ROWS * W, T], DT_X, kind="ExternalInput").ap()
    wts = nc.dram_tensor("wts", [W, 8 * 128], DT_W, kind="ExternalInput").ap()
    btab = nc.dram_tensor("btab", [128, 17], F32, kind="ExternalInput").ap()
    outH = nc.dram_tensor("outH", [ROWS * 128, T], DT_E, kind="ExternalOutput").ap()

    n_sb = T // SB_SEG
    n_blk = SB_SEG // BLK
    relu = mybir.ActivationFunctionType.Relu
    op_max, op_add = mybir.AluOpType.max, mybir.AluOpType.add
    act_pairs = list(RELU_ACT_PAIRS)
    chain_pairs = [j for j in range(8) if j not in RELU_ACT_PAIRS]
    assert len(act_pairs) == 4, "interleaved tree assumes 4 ScalarE pairs"

    with TileContext(nc) as tc:
        with (
            tc.tile_pool(name="consts", bufs=1) as cpool,
            tc.tile_pool(name="xin", bufs=3) as xpool,
            tc.tile_pool(name="psum", bufs=4, space="PSUM") as pspool,
            tc.tile_pool(name="relu", bufs=2 * len(act_pairs) * n_blk + 4) as rpool,
            tc.tile_pool(name="chain", bufs=4 * n_blk) as chpool,
            tc.tile_pool(name="tmp", bufs=12) as tpool,
        ):
            w_sb = cpool.tile([W, 8 * 128], DT_W)
            nc.sync.dma_start(w_sb[:], wts[:])
            b_sb = cpool.tile([128, 17], F32)
            nc.sync.dma_start(b_sb[:], btab[:])
            # cols 0..7: +b/16 per pair; 8..15: -b/16; 16: chain-bias constant

            # warm each compute engine's view of b_sb so later ops carry
            # fewer distinct sync waits (walrus caps waits per instruction)
            warm = cpool.tile([128, 17], F32)
            nc.vector.tensor_copy(out=warm[:], in_=b_sb[:])
            warm2 = cpool.tile([128, 17], F32)
            nc.gpsimd.tensor_copy(out=warm2[:], in_=b_sb[:])
            warm3 = cpool.tile([128, 17], F32)
            nc.scalar.copy(warm3[:], b_sb[:])

            for r in range(ROWS):
                for sb in range(n_sb):
                    s0 = sb * SB_SEG
                    x_sb = xpool.tile([W, SB_SEG], DT_X)
                    nc.sync.dma_start(
                        x_sb[:], xwin[r * W:(r + 1) * W, s0:s0 + SB_SEG])
                    # interleave chain/ACT pairs so VectorE and ScalarE both
                    # start early; chain PSUM tiles are consumed immediately.
                    order = []
                    for i in range(max(len(chain_pairs), len(act_pairs))):
                        if i < len(chain_pairs):
                            order.append(chain_pairs[i])
                        if i < len(act_pairs):
                            order.append(act_pairs[i])
                    accs = [None] * n_blk
                    rts = [[] for _ in range(n_blk)]
                    for j in order:
                        lhsT = w_sb[:, j * 128:(j + 1) * 128]
                        if CONV_DT == "fp32r":
                            lhsT = lhsT.bitcast(mybir.dt.float32r)
                        for blk in range(n_blk):
                            ps = pspool.tile([128, BLK], F32)
                            for m in range(BLK // MMN):
                                rhs = x_sb[:, blk * BLK + m * MMN:
                                           blk * BLK + (m + 1) * MMN]
                                if CONV_DT == "fp32r":
                                    rhs = rhs.bitcast(mybir.dt.float32r)
                                nc.tensor.matmul(
                                    ps[:, m * MMN:(m + 1) * MMN], lhsT, rhs,
                                    start=True, stop=True)
                            if j in RELU_ACT_PAIRS:
                                rt = rpool.tile([128, BLK], DT_E)
                                nc.scalar.activation(
                                    rt[:], ps[:], relu, bias=b_sb[:, j:j + 1])
                                rts[blk].append(rt)
                            elif accs[blk] is None:
                                acc = chpool.tile([128, BLK], DT_E, tag="acc")
                                nc.vector.tensor_scalar(
                                    acc[:], ps[:], b_sb[:, 8 + j:9 + j], None,
                                    op_max)
                                accs[blk] = acc
                            else:
                                nacc = chpool.tile([128, BLK], DT_E, tag="acc")
                                nc.vector.scalar_tensor_tensor(
                                    nacc[:], ps[:], b_sb[:, 8 + j:9 + j],
                                    accs[blk][:], op_max, op_add)
                                accs[blk] = nacc
                    for blk in range(n_blk):
                        # tree-add the ScalarE relu tiles; engine alternates by
                        # block parity to split load between gpsimd and vector
                        teng = nc.gpsimd if blk % 2 == 0 else nc.vector
                        cur = rts[blk]
                        while len(cur) > 1:
                            nxt = []
                            for i in range(0, len(cur) - 1, 2):
                                o = tpool.tile([128, BLK], DT_E)
                                teng.tensor_tensor(
                                    o[:], cur[i][:], cur[i + 1][:], op_add)
                                nxt.append(o)
                            if len(cur) % 2:
                                nxt.append(cur[-1])
                            cur = nxt
                        top = tpool.tile([128, BLK], DT_E, tag="top")
                        teng.tensor_tensor(
                            top[:], cur[0][:], accs[blk][:], op_add)
                        # halves fold (+ chain-bias constant) happen on host
                        nc.sync.dma_start(
                            outH[r * 128:(r + 1) * 128, s0 + blk * BLK:
                                 s0 + (blk + 1) * BLK], top[:])
    nc.compile()
    return nc


_NC_CACHE = None


def _get_nc():
    global _NC_CACHE
    if _NC_CACHE is None:
        _NC_CACHE = _build_nc()
    return _NC_CACHE


LAST_RESULTS = None


def _install_ntff_hook():
    """Provide antenv.axon_hooks (absent on this image) so
    run_bass_kernel_spmd(trace=True) can capture NTFF profiles via the
    axon PJRT plugin's C ABI. Also stub the artifact upload (no bucket
    creds in-container)."""
    import contextlib
    import ctypes
    import sys
    import types

    try:
        from antenv.axon_hooks import get_axon_ntff_profile_hook  # noqa: F401
        return  # real module present
    except ImportError:
        pass

    so_path = "/opt/axon/libaxon_pjrt.so"
    lib = ctypes.CDLL(so_path)
    lib.axon_start_nrt_profile.argtypes = [
        ctypes.POINTER(ctypes.c_int64), ctypes.c_size_t]
    lib.axon_start_nrt_profile.restype = ctypes.c_int64
    lib.axon_stop_nrt_profile.argtypes = [ctypes.c_char_p]
    lib.axon_stop_nrt_profile.restype = ctypes.c_int64

    @contextlib.contextmanager
    def _hook(output_dir, device_ids):
        import jax
        jax.devices()
        if device_ids:
            ids = (ctypes.c_int64 * len(device_ids))(*device_ids)
            rc = lib.axon_start_nrt_profile(ids, len(device_ids))
        else:
            rc = lib.axon_start_nrt_profile(None, 0)
        if rc != 0:
            raise RuntimeError(f"axon_start_nrt_profile rc={rc}")
        try:
            yield
        finally:
            n = lib.axon_stop_nrt_profile(str(output_dir).encode())
            print(f"ntff profile: {n} file(s) -> {output_dir}")

    mod = types.ModuleType("antenv.axon_hooks")
    mod.get_axon_ntff_profile_hook = lambda: _hook
    mod.set_axon_ntff_profile_hook = lambda h: None
    sys.modules["antenv.axon_hooks"] = mod
    bass_utils.upload_artifacts = lambda tmpdir: f"file://{tmpdir}"


def host_inputs(x, w, b):
    """Build the 8 per-core input maps from the full problem inputs."""
    x = np.asarray(x, np.float32)
    xpad = np.pad(x[:, 0, :], ((0, 0), (HALO, HALO)))  # [B, L+32]
    s = xpad.strides
    np_x = mybir.dt.np(DT_X)
    xwinT = np.lib.stride_tricks.as_strided(
        xpad, shape=(B, W, T), strides=(s[0], s[1], P * s[1]))

    lhsT = _build_lhsT(w).astype(mybir.dt.np(DT_W))
    bsc = np.asarray(b, np.float32) / 16.0
    btab = np.zeros((128, 17), dtype=np.float32)
    for j in range(8):
        col = np.concatenate([np.full(P, bsc[2 * j]), np.full(P, bsc[2 * j + 1])])
        btab[:, j] = col
        btab[:, 8 + j] = -col
    chain_pairs = [j for j in range(8) if j not in RELU_ACT_PAIRS]
    cb = sum(float(bsc[2 * j] + bsc[2 * j + 1]) for j in chain_pairs[1:])
    btab[:, 16] = cb

    in_maps = []
    for core in range(NCORES):
        rows = xwinT[core * ROWS:(core + 1) * ROWS]          # [4, 96, T]
        in_maps.append({
            "xwin": np.ascontiguousarray(rows, dtype=np_x).reshape(ROWS * W, T),
            "wts": lhsT,
            "btab": btab,
        })
    return in_maps


def kernel(x, w, b):
    global LAST_RESULTS
    in_maps = host_inputs(x, w, b)
    nc = _get_nc()
    trace = bool(os.environ.get("KERNEL_TRACE"))
    if trace:
        _install_ntff_hook()
    res = bass_utils.run_bass_kernel_spmd(
        nc, in_maps, core_ids=list(range(NCORES)), trace=trace,
        **({"trace_cores": [0]} if trace else {}),
    )
    LAST_RESULTS = res

    chain_pairs = [j for j in range(8) if j not in RELU_ACT_PAIRS]
    bsc = np.asarray(b, np.float32) / 16.0
    cb = sum(float(bsc[2 * j] + bsc[2 * j + 1]) for j in chain_pairs)
    out = np.empty((B, L), dtype=np.float32)
    for core in range(NCORES):
        oH = res.results[core]["outH"].reshape(ROWS, 2, P, T).astype(np.float32)
        folded = oH[:, 0] + oH[:, 1] + cb                      # [ROWS, P, T]
        for r in range(ROWS):
            out[core * ROWS + r] = folded[r].T.reshape(L)
    return out

